# revision 4
# baseline (speedup 1.0000x reference)
"""VQ codebook (nn_Codebook) Trainium2 kernel.

Data-parallel over 8 NeuronCores: each core takes 4 batches of z
(z[b] is [C=256, H*W=1024], already contraction-major), the codebook is
replicated. Per core:
  psum[n,k] = sum_d z[d,n] * (2*emb[k,d])          (PE, float32r)
  m[n,k]    = fl(psum - ||z_n||^2)                 (ACT Identity bias-copy, = -d)
  idx[n]    = first argmax_k m                     (DVE max8 + max_index)
  e         = emb[idx]                             (indirect DMA gather)
  z_q       = fl(z + fl(e - z))                    (straight-through, bitwise)
  loss_part = sum fl(e - z)^2                      (DVE tensor_tensor_reduce)
Host sums loss partials, reshapes z_q, casts idx to int32.

The fp32 reference quantizes distances to ulp(||z||^2) (~3e-5), so the
argmin is dominated by that rounding; emulating d = fl(zsq - 2*z.e) with
first-index tie-break reproduces the reference indices exactly
(||e||^2 < ulp(zsq)/2 is absorbed and never affects the result).
"""

import sys

for _p in ("/opt/trn_rl_repo", "/root/.axon_site/_ro/trn_rl_repo"):
    if _p not in sys.path:
        sys.path.append(_p)

import numpy as np

import concourse.bacc as bacc
import concourse.mybir as mybir
import concourse.tile as tile
from concourse import bass_utils
from concourse.bass import IndirectOffsetOnAxis
from concourse.masks import make_identity

N_CORES = 8
B = 32          # total batches
C = 256         # latent dim (contraction)
HW = 1024       # points per batch
K = 8192        # codebook size
KC = 512        # codebook chunk (psum bank / fp32 moving max)
NT = 128        # points per tile
BPC = B // N_CORES          # batches per core
TILES = BPC * HW // NT      # 32 tiles per core
NKC = K // KC               # 16 chunks
F32 = mybir.dt.float32
F32R = mybir.dt.float32r
U32 = mybir.dt.uint32


def build(repeat: int = 1):
    """Build the per-core Bass program. repeat>1 wraps the tile loop in a
    hardware For_i for benchmarking (same data each iteration)."""
    nc = bacc.Bacc("TRN2", target_bir_lowering=False, debug=False)

    zt = nc.dram_tensor("zt", [BPC, C, HW], F32R, kind="ExternalInput")
    embt2 = nc.dram_tensor("embt2", [C, K], F32R, kind="ExternalInput")
    emb = nc.dram_tensor("emb", [K, C], F32, kind="ExternalInput")
    nzsq = nc.dram_tensor("nzsq", [BPC, HW, 1], F32, kind="ExternalInput")

    idx_out = nc.dram_tensor("idx_out", [BPC, HW, 8], U32, kind="ExternalOutput")
    zqt_out = nc.dram_tensor("zqt_out", [BPC, C, HW], F32, kind="ExternalOutput")
    loss_out = nc.dram_tensor("loss_out", [128, 1], F32, kind="ExternalOutput")

    with tile.TileContext(nc) as tc:
        with (
            tc.tile_pool(name="const", bufs=1) as const_pool,
            tc.tile_pool(name="zin", bufs=3) as zin_pool,
            tc.tile_pool(name="mbuf", bufs=2) as m_pool,
            tc.tile_pool(name="small", bufs=3) as small_pool,
            tc.tile_pool(name="ste", bufs=3) as ste_pool,
            tc.tile_pool(name="mm", bufs=6, space="PSUM") as mm_pool,
            tc.tile_pool(name="tpsum", bufs=2, space="PSUM") as tp_pool,
        ):
            # resident: codebook (transposed, pre-scaled by 2), identity
            eb0 = const_pool.tile([128, K], F32R, tag="eb0")
            eb1 = const_pool.tile([128, K], F32R, tag="eb1")
            nc.sync.dma_start(eb0[:], embt2.ap()[0:128, :])
            nc.sync.dma_start(eb1[:], embt2.ap()[128:256, :])
            ident = const_pool.tile([128, 128], F32, tag="ident")
            make_identity(nc, ident[:])
            loss_acc = const_pool.tile([128, 1], F32, tag="loss_acc")
            nc.vector.memset(loss_acc[:], 0.0)

            def tile_body(t):
                b, n0 = t // (HW // NT), (t % (HW // NT)) * NT
                z0 = zin_pool.tile([128, NT], F32R, tag="z0")
                z1 = zin_pool.tile([128, NT], F32R, tag="z1")
                nc.sync.dma_start(z0[:], zt.ap()[b, 0:128, n0 : n0 + NT])
                nc.sync.dma_start(z1[:], zt.ap()[b, 128:256, n0 : n0 + NT])
                zb = small_pool.tile([128, 1], F32, tag="zb")
                nc.sync.dma_start(zb[:], nzsq.ap()[b, n0 : n0 + NT, :])

                m = m_pool.tile([128, K], F32, tag="m")
                for g in range(NKC):
                    ps = mm_pool.tile([128, KC], F32, tag="ps")
                    ksl = slice(g * KC, (g + 1) * KC)
                    nc.tensor.matmul(ps[:], z0[:], eb0[:, ksl], start=True, stop=False)
                    nc.tensor.matmul(ps[:], z1[:], eb1[:, ksl], start=False, stop=True)
                    # m = fl(psum - zsq)  (exact fp32 add of per-partition bias)
                    nc.scalar.activation(
                        m[:, ksl], ps[:], mybir.ActivationFunctionType.Identity,
                        bias=zb[:], scale=1.0,
                    )

                maxv = small_pool.tile([128, 8], F32, tag="maxv")
                nc.vector.max(maxv[:], m[:])
                midx = small_pool.tile([128, 8], U32, tag="midx")
                nc.vector.max_index(midx[:], maxv[:], m[:])

                nc.sync.dma_start(idx_out.ap()[b, n0 : n0 + NT, :], midx[:])

                eg = ste_pool.tile([128, C], F32, tag="eg")
                nc.gpsimd.indirect_dma_start(
                    out=eg[:],
                    out_offset=None,
                    in_=emb.ap(),
                    in_offset=IndirectOffsetOnAxis(ap=midx[:, 0:1], axis=0),
                )

                for cbase, zc in ((0, z0), (128, z1)):
                    tr = tp_pool.tile([128, 128], F32, tag="tr")
                    nc.tensor.transpose(tr[:], eg[:, cbase : cbase + 128], ident[:])
                    # DVE PSUM reads are lossy; ACT copy is bit-exact
                    trs = ste_pool.tile([128, 128], F32, tag="trs")
                    nc.scalar.copy(trs[:], tr[:])
                    tdiff = ste_pool.tile([128, NT], F32, tag="tdiff")
                    # t = fl(e - z); z tiles are f32r but bits are f32
                    nc.gpsimd.tensor_sub(tdiff[:], trs[:], zc[:].bitcast(F32))
                    zq = ste_pool.tile([128, NT], F32, tag="zq")
                    nc.gpsimd.tensor_add(zq[:], zc[:].bitcast(F32), tdiff[:])
                    nc.sync.dma_start(
                        zqt_out.ap()[b, cbase : cbase + 128, n0 : n0 + NT], zq[:]
                    )
                    sq = ste_pool.tile([128, NT], F32, tag="sq")
                    lp = small_pool.tile([128, 1], F32, tag="lp")
                    nc.vector.tensor_mul(sq[:], tdiff[:], tdiff[:])
                    nc.vector.reduce_sum(lp[:], sq[:], axis=mybir.AxisListType.X)
                    nc.vector.tensor_add(loss_acc[:], loss_acc[:], lp[:])

            if repeat == 1:
                for t in range(TILES):
                    tile_body(t)
            else:
                with tc.For_i(0, repeat, 1):
                    for t in range(TILES):
                        tile_body(t)

            nc.sync.dma_start(loss_out.ap(), loss_acc[:])

    nc.compile()
    return nc


_CACHED = {}


def _get(repeat=1):
    if repeat not in _CACHED:
        _CACHED[repeat] = build(repeat)
    return _CACHED[repeat]


def prepare_inputs(z, emb):
    z = np.ascontiguousarray(z, dtype=np.float32)
    emb = np.ascontiguousarray(emb, dtype=np.float32)
    zr = z.reshape(B, C, HW)
    embt2 = np.ascontiguousarray((emb * np.float32(2.0)).T)
    nzsq = -np.square(zr).sum(axis=1, dtype=np.float32)  # [B, HW]
    in_maps = []
    for c in range(N_CORES):
        bs = slice(c * BPC, (c + 1) * BPC)
        in_maps.append(
            {
                "zt": np.ascontiguousarray(zr[bs]),
                "embt2": embt2,
                "emb": emb,
                "nzsq": np.ascontiguousarray(nzsq[bs])[..., None],
            }
        )
    return in_maps


def postprocess(results, z, emb):
    cand = np.concatenate(
        [r["idx_out"].reshape(-1, 8) for r in results]
    ).astype(np.int64)                                   # [N, 8] fp32r top-8
    zq = np.concatenate([r["zqt_out"] for r in results], axis=0)  # [B, C, HW]
    total = np.float64(0.0)
    for r in results:
        total += np.float64(r["loss_out"].sum(dtype=np.float64))

    # Host arbitration: fp32r ranking can flip near-ties; recompute the exact
    # fp32 distance for the top-8 candidates and re-argmin with the
    # reference's first-index tie-break (d is quantized to ulp(zsq)).
    zr = z.reshape(B, C, HW)
    zf = zr.transpose(0, 2, 1).reshape(-1, C)            # [N, C] f32
    zsq = np.square(zf).sum(axis=1, dtype=np.float32)
    e2c = emb[cand] * np.float32(2.0)                    # [N, 8, C]
    dot = np.einsum("nc,nkc->nk", zf, e2c, dtype=np.float32, casting="same_kind")
    dcand = (zsq[:, None] - dot).astype(np.float32)
    dmin = dcand.min(axis=1, keepdims=True)
    idx_arb = np.where(dcand == dmin, cand, np.int64(1 << 40)).min(axis=1)
    idx_dev = cand[:, 0]
    changed = np.nonzero(idx_arb != idx_dev)[0]
    idx = idx_arb.astype(np.int32)

    if changed.size:
        zq_pts = zq.reshape(B, C, HW)
        for n in changed:
            b, hw = divmod(int(n), HW)
            zrow = zf[n]
            t_old = (emb[idx_dev[n]] - zrow).astype(np.float32)
            t_new = (emb[idx_arb[n]] - zrow).astype(np.float32)
            zq_pts[b, :, hw] = (zrow + t_new).astype(np.float32)
            total += np.square(t_new, dtype=np.float64).sum() - np.square(
                t_old, dtype=np.float64
            ).sum()

    zq = zq.reshape(B, C, 32, 32)
    loss = np.float32(np.float32(total) / np.float32(B * HW * C) * np.float32(1.25))
    return zq, idx, loss


def kernel(z, emb):
    z = np.ascontiguousarray(z, dtype=np.float32)
    emb = np.ascontiguousarray(emb, dtype=np.float32)
    nc = _get(1)
    in_maps = prepare_inputs(z, emb)
    res = bass_utils.run_bass_kernel_spmd(nc, in_maps, core_ids=list(range(N_CORES)))
    return postprocess(res.results, z, emb)
